# revision 23
# baseline (speedup 1.0000x reference)
"""Trainium2 Bass kernel for nn_Attention_22342419874323.

Distribution: 8 cores; core c handles batch b = c//2, way w = c%2
(the NFR=8 fragments of one (b, w) pair) -> perfectly data-parallel.

Core algorithm: for each output channel v build a scaled stationary
    A_v[d, q] = qprojT[d, q] * qfW[d, v] + fW[d, v]      (fp16)
so that  matmul(A_v, fragprojT)[q, rl] = qf3[l, q, v] + f_att[l, v]
(the f_att term folds in because sum_d fW[d,v] fragprojT[d,rl] =
f_att[rl,v]).  PSUM holds [q, (r, l)]; the max over l is a free-dim
reduction:
 - "direct" v-groups: DVE tensor_reduce(max) straight from PSUM
 - other groups: ACT copies PSUM -> fp16 SBUF (2 v per op), then DVE
   runs a 2x-packed TT-max tree over 16 v at a time.
gate/val logits then need only two dot products over v, batched on DVE.

Host (numpy) does all transposes / broadcast constants / blob packing;
device does all model FLOPs.
"""

import sys

sys.path.insert(0, "/opt/trn_rl_repo")

import numpy as np

import concourse.bass as bass
import concourse.mybir as mybir
import concourse.tile as tile
from concourse import bacc
from concourse.bass_utils import run_bass_kernel_spmd

B, LQ, LF, NW, NFR, E, D, V = 4, 128, 64, 2, 8, 128, 128, 64
NF = NW * NFR
RL = NFR * LF  # 512
F32 = mybir.dt.float32
F32R = mybir.dt.float32r
F16 = mybir.dt.float16
AF = mybir.ActivationFunctionType
OP = mybir.AluOpType
X = mybir.AxisListType.X

# hot blob (f32 cols): qT 128 | projb 1 | qW 64 | qfW 64 | fW 64 |
#                      qmask 1 | ones_c 1 | selfw16 1
HOT = {"qT": (0, 128), "projb": (128, 1), "qW": (129, 64),
       "qfW": (193, 64), "fW": (257, 64), "qmask": (321, 1),
       "ones_c": (322, 1), "selfw": (323, 1)}
NHOT = 324
# cold blob (f32 cols): ident 128 | gB 64 | vB 64 | gB8 512 | vB8 512
COLD = {"ident": (0, 128), "gB": (128, 64), "vB": (192, 64),
        "gB8": (256, 512), "vB8": (768, 512)}
NCOLD = 1280
# row blob [1, 640]: fmask 512 | ones_r 128
ROW = {"fmask": (0, 512), "ones_r": (512, 128)}
NROW = 640

_CACHE = {}


def _build():
    nc = bacc.Bacc(None, target_bir_lowering=False, debug=False)

    d_hot = nc.dram_tensor("hot", [128, NHOT], F32, kind="ExternalInput")
    d_cold = nc.dram_tensor("cold", [128, NCOLD], F32, kind="ExternalInput")
    d_row = nc.dram_tensor("rowb", [1, NROW], F32, kind="ExternalInput")
    d_projW = nc.dram_tensor("projW", [E, D], F32R, kind="ExternalInput")
    d_fragT = nc.dram_tensor("fragT", [E, RL], F32R, kind="ExternalInput")

    do = {}
    for name, shape in [
        ("o_fragcodeT", [D, NFR]),
        ("o_qcode", [NFR, D]),
        ("o_fratt", [1, RL]),
        ("o_gate", [LQ, NFR]),
        ("o_query", [LQ, D]),
    ]:
        do[name] = nc.dram_tensor(name, shape, F32, kind="ExternalOutput")

    with tile.TileContext(nc) as tc:
        with (
            tc.tile_pool(name="consts", bufs=1) as cpool,
            tc.tile_pool(name="work", bufs=2) as wpool,
            tc.tile_pool(name="av", bufs=6) as apool,
            tc.tile_pool(name="stgp", bufs=3) as stgpool,
            tc.tile_pool(name="psbig", bufs=3, space="PSUM") as psb,
            tc.tile_pool(name="pssm", bufs=2, space="PSUM") as pss,
        ):
            # ---- consolidated input DMAs ----
            t_hot = cpool.tile([128, NHOT], F32, tag="hot")
            t_cold = cpool.tile([128, NCOLD], F32, tag="cold")
            t_row = cpool.tile([1, NROW], F32, tag="row")
            t_projW = cpool.tile([128, D], F32R, tag="projW")
            t_fragT = cpool.tile([128, RL], F32R, tag="fragT")
            nc.sync.dma_start(out=t_hot[:], in_=d_hot[:])
            nc.sync.dma_start(out=t_projW[:], in_=d_projW[:])
            nc.scalar.dma_start(out=t_fragT[:], in_=d_fragT[:])
            nc.scalar.dma_start(out=t_cold[:], in_=d_cold[:])
            nc.sync.dma_start(out=t_row[:], in_=d_row[:])

            def hot(name):
                o, n = HOT[name]
                return t_hot[:, o:o + n]

            def cold(name):
                o, n = COLD[name]
                return t_cold[:, o:o + n]

            def row(name):
                o, n = ROW[name]
                return t_row[:, o:o + n]

            sw16 = hot("selfw").bitcast(F16)[:, 0:1]

            # ---- projections ----
            ps_qpT = pss.tile([128, 128], F32, tag="psq")
            nc.tensor.matmul(ps_qpT[:], lhsT=t_projW[:].bitcast(F32),
                             rhs=hot("qT"), start=True, stop=True)
            sb_qprojT = cpool.tile([128, 128], F32, tag="qprojT")
            nc.vector.tensor_scalar(sb_qprojT[:], ps_qpT[:], hot("projb"),
                                    None, OP.add)
            sb_qprojT16 = cpool.tile([128, 128], F16, tag="qprojT16")
            nc.vector.tensor_copy(sb_qprojT16[:], sb_qprojT[:])

            ps_fpT = psb.tile([128, 2 * RL], F32, tag="psbig")
            nc.tensor.matmul(ps_fpT[:, 0:RL], lhsT=t_projW[:],
                             rhs=t_fragT[:], start=True, stop=True)
            sb_fragprojT = cpool.tile([128, RL], F16, tag="fragprojT")
            nc.vector.tensor_scalar(sb_fragprojT[:], ps_fpT[:, 0:RL],
                                    hot("projb"), None, OP.add)

            # qproj natural [q, d] via PE transpose (+ o_query)
            ps_qn = pss.tile([128, 128], F32, tag="psq")
            nc.tensor.transpose(ps_qn[:], sb_qprojT[:], cold("ident"))
            sb_qprojN = cpool.tile([128, 128], F32, tag="qprojN")
            nc.scalar.copy(sb_qprojN[:], ps_qn[:])
            nc.sync.dma_start(out=do["o_query"][:], in_=sb_qprojN[:])

            # q_att[q, v]
            ps_qa = pss.tile([128, V], F32, tag="psq")
            nc.tensor.matmul(ps_qa[:], lhsT=sb_qprojT[:], rhs=hot("qW"),
                             start=True, stop=True)
            sb_qatt = cpool.tile([128, V], F32, tag="qatt")
            nc.scalar.copy(sb_qatt[:], ps_qa[:])

            # ---- fragment self-attention ----
            ps_fl = pss.tile([1, RL], F32, tag="psq")
            nc.tensor.matmul(ps_fl[:], lhsT=sw16, rhs=sb_fragprojT[:],
                             start=True, stop=True)
            sb_e = wpool.tile([1, RL], F32, tag="fr_e")
            nc.scalar.activation(sb_e[:], ps_fl[:], AF.Exp)
            sb_em = wpool.tile([1, RL], F32, tag="fr_em")
            nc.vector.tensor_tensor(sb_em[:], sb_e[:], row("fmask"), OP.mult)
            sb_sums = wpool.tile([1, NFR], F32, tag="fr_sums")
            nc.vector.tensor_reduce(
                sb_sums[:], sb_em[:].rearrange("p (r l) -> p r l", r=NFR),
                axis=X, op=OP.add)
            sb_rec = wpool.tile([1, NFR], F32, tag="fr_rec")
            nc.vector.tensor_scalar(sb_rec[:], sb_sums[:], 1e-7, None, OP.add)
            sb_rec2 = wpool.tile([1, NFR], F32, tag="fr_rec2")
            nc.vector.reciprocal(sb_rec2[:], sb_rec[:])
            sb_att = wpool.tile([1, RL], F32, tag="fr_att")
            for r in range(NFR):
                nc.vector.tensor_scalar(
                    sb_att[0:1, r * LF:(r + 1) * LF],
                    sb_em[0:1, r * LF:(r + 1) * LF],
                    sb_rec2[0:1, r:r + 1], None, OP.mult)
            nc.sync.dma_start(out=do["o_fratt"][:], in_=sb_att[:])

            # ---- main loop over v ----
            DIRECT = set()
            sb_M = cpool.tile([128, V * NFR], F32, tag="M")

            def emit_group(g):
                direct = g in DIRECT
                sb_A8 = apool.tile([128, 8 * 128], F16, tag="av")
                eng = nc.vector if direct else nc.gpsimd
                for j in range(8):
                    v = g * 8 + j
                    eng.tensor_scalar(sb_A8[:, j * 128:(j + 1) * 128],
                                      sb_qprojT16[:],
                                      hot("qfW")[:, v:v + 1],
                                      hot("fW")[:, v:v + 1],
                                      OP.mult, OP.add)
                stage_f16 = None
                if not direct:
                    stage_f16 = stgpool.tile([128, 8 * RL], F16,
                                             tag="stg")
                for j2 in range(4):
                    ps_v = psb.tile([128, 2 * RL], F32, tag="psbig")
                    for k in range(2):
                        j = j2 * 2 + k
                        nc.tensor.matmul(
                            ps_v[:, k * RL:(k + 1) * RL],
                            lhsT=sb_A8[:, j * 128:(j + 1) * 128],
                            rhs=sb_fragprojT[:],
                            start=True, stop=True)
                    if direct:
                        nc.vector.tensor_reduce(
                            sb_M[:, (g * 8 + j2 * 2) * NFR:
                                 (g * 8 + j2 * 2 + 2) * NFR],
                            ps_v[:].rearrange("q (w f l) -> q (w f) l",
                                              w=2, f=NFR),
                            axis=X, op=OP.max)
                    else:
                        nc.scalar.copy(
                            stage_f16[:, j2 * 2 * RL:(j2 + 1) * 2 * RL],
                            ps_v[:])
                return stage_f16

            def emit_tree(g0, b0, g1, b1):
                eng = nc.vector
                width = LF
                cur = None
                while width > 2:
                    half = width // 2
                    nxt_t = wpool.tile([128, 128 * half], F16,
                                       tag=f"tr{half}")
                    nxt = nxt_t[:].rearrange("q (g l) -> q g l", g=128)
                    if cur is None:
                        c0 = b0[:].rearrange("q (g l) -> q g l", g=64)
                        c1 = b1[:].rearrange("q (g l) -> q g l", g=64)
                        eng.tensor_tensor(nxt[:, 0:64, :], c0[:, :, 0:half],
                                          c0[:, :, half:width], OP.max)
                        eng.tensor_tensor(nxt[:, 64:128, :], c1[:, :, 0:half],
                                          c1[:, :, half:width], OP.max)
                    else:
                        eng.tensor_tensor(nxt, cur[:, :, 0:half],
                                          cur[:, :, half:width], OP.max)
                    cur = nxt
                    width = half
                eng.tensor_tensor(
                    sb_M[:, g0 * 64:(g0 + 1) * 64]
                    .rearrange("q (g l) -> q g l", g=64),
                    cur[:, 0:64, 0:1], cur[:, 0:64, 1:2], OP.max)
                eng.tensor_tensor(
                    sb_M[:, g1 * 64:(g1 + 1) * 64]
                    .rearrange("q (g l) -> q g l", g=64),
                    cur[:, 64:128, 0:1], cur[:, 64:128, 1:2], OP.max)

            pend = []
            for g in range(V // 8):
                buf = emit_group(g)
                if buf is not None:
                    pend.append((g, buf))
                if len(pend) == 2:
                    (g0, b0), (g1, b1) = pend
                    emit_tree(g0, b0, g1, b1)
                    pend = []

            # ---- gate / val logits ----
            sb_tmp64 = wpool.tile([128, V], F32, tag="tmp64")
            sb_cg = wpool.tile([128, 1], F32, tag="cg")
            sb_cv = wpool.tile([128, 1], F32, tag="cv")
            nc.vector.tensor_tensor(sb_tmp64[:], sb_qatt[:], cold("gB"),
                                    OP.mult)
            nc.vector.tensor_reduce(sb_cg[:], sb_tmp64[:], axis=X, op=OP.add)
            nc.vector.tensor_tensor(sb_tmp64[:], sb_qatt[:], cold("vB"),
                                    OP.mult)
            nc.vector.tensor_reduce(sb_cv[:], sb_tmp64[:], axis=X, op=OP.add)

            M3 = sb_M[:].rearrange("q (v f) -> q f v", v=V)
            gB83 = cold("gB8").rearrange("q (f v) -> q f v", f=NFR)
            vB83 = cold("vB8").rearrange("q (f v) -> q f v", f=NFR)
            sb_prodG = wpool.tile([128, NFR * V], F32, tag="prodG")
            sb_glogr = wpool.tile([128, NFR], F32, tag="glogr")
            sb_glog = wpool.tile([128, NFR], F32, tag="glog")
            nc.vector.tensor_tensor(
                sb_prodG[:].rearrange("q (f v) -> q f v", f=NFR), M3, gB83,
                OP.mult)
            nc.vector.tensor_reduce(
                sb_glogr[:], sb_prodG[:].rearrange("q (f v) -> q f v", f=NFR),
                axis=X, op=OP.add)
            nc.vector.tensor_scalar(sb_glog[:], sb_glogr[:], sb_cg[:, 0:1],
                                    None, OP.add)
            sb_prodV = wpool.tile([128, NFR * V], F32, tag="prodV")
            sb_vlogr = wpool.tile([128, NFR], F32, tag="vlogr")
            sb_vlog = wpool.tile([128, NFR], F32, tag="vlog")
            nc.vector.tensor_tensor(
                sb_prodV[:].rearrange("q (f v) -> q f v", f=NFR), M3, vB83,
                OP.mult)
            nc.vector.tensor_reduce(
                sb_vlogr[:], sb_prodV[:].rearrange("q (f v) -> q f v", f=NFR),
                axis=X, op=OP.add)
            nc.vector.tensor_scalar(sb_vlog[:], sb_vlogr[:], sb_cv[:, 0:1],
                                    None, OP.add)

            # frag_code[d, r] = sum_l fragprojT[d, rl] * att[rl]
            ps_attB = pss.tile([128, RL], F32, tag="psq")
            nc.tensor.matmul(ps_attB[:], lhsT=row("ones_r"), rhs=sb_att[:],
                             start=True, stop=True)
            sb_ab = wpool.tile([128, RL], F32, tag="ab")
            nc.scalar.copy(sb_ab[:], ps_attB[:])
            sb_prod = wpool.tile([128, RL], F32, tag="prod")
            nc.vector.tensor_tensor(sb_prod[:], sb_fragprojT[:], sb_ab[:],
                                    OP.mult)
            sb_fcT = wpool.tile([128, NFR], F32, tag="fcT")
            nc.vector.tensor_reduce(
                sb_fcT[:], sb_prod[:].rearrange("p (r l) -> p r l", r=NFR),
                axis=X, op=OP.add)
            nc.sync.dma_start(out=do["o_fragcodeT"][:], in_=sb_fcT[:])

            # gate = sigmoid(glog) * qmask
            sb_sig = wpool.tile([128, NFR], F32, tag="sig")
            nc.scalar.activation(sb_sig[:], sb_glog[:], AF.Sigmoid)
            sb_gate = wpool.tile([128, NFR], F32, tag="gate")
            nc.vector.tensor_scalar(sb_gate[:], sb_sig[:], hot("qmask"),
                                    None, OP.mult)
            nc.sync.dma_start(out=do["o_gate"][:], in_=sb_gate[:])

            # val softmax over q (partition sum via ones matmul)
            sb_e2 = wpool.tile([128, NFR], F32, tag="e2")
            nc.scalar.activation(sb_e2[:], sb_vlog[:], AF.Exp)
            sb_e2m = wpool.tile([128, NFR], F32, tag="e2m")
            nc.vector.tensor_scalar(sb_e2m[:], sb_e2[:], hot("qmask"),
                                    None, OP.mult)
            ps_s = pss.tile([1, NFR], F32, tag="psq")
            nc.tensor.matmul(ps_s[:], lhsT=hot("ones_c"), rhs=sb_e2m[:],
                             start=True, stop=True)
            sb_s = wpool.tile([1, NFR], F32, tag="s")
            nc.vector.tensor_scalar(sb_s[:], ps_s[:], 1e-7, None, OP.add)
            sb_r2 = wpool.tile([1, NFR], F32, tag="r2")
            nc.vector.reciprocal(sb_r2[:], sb_s[:])
            ps_rb = pss.tile([128, NFR], F32, tag="psq")
            nc.tensor.matmul(ps_rb[:], lhsT=row("ones_r"), rhs=sb_r2[:],
                             start=True, stop=True)
            sb_qfn = wpool.tile([128, NFR], F32, tag="qfn")
            nc.vector.tensor_tensor(sb_qfn[:], sb_e2m[:], ps_rb[:], OP.mult)

            # query_code[f, d] = sum_q qfn[q, f] * qprojN[q, d]
            ps_qc = pss.tile([NFR, 128], F32, tag="psq")
            nc.tensor.matmul(ps_qc[:], lhsT=sb_qfn[:], rhs=sb_qprojN[:],
                             start=True, stop=True)
            sb_qc = wpool.tile([NFR, 128], F32, tag="qc")
            nc.scalar.copy(sb_qc[:], ps_qc[:])
            nc.sync.dma_start(out=do["o_qcode"][:], in_=sb_qc[:])

    nc.compile()
    return nc


def _get_nc():
    if "nc" not in _CACHE:
        _CACHE["nc"] = _build()
    return _CACHE["nc"]


def _make_in_maps(query, fragment, query_mask, fragment_mask, proj_W, proj_b,
                  self_att_W, q_att_W, f_att_W, qf_att_W, gate_w, val_w):
    cold = np.zeros((128, NCOLD), np.float32)
    cold[:, 0:128] = np.eye(128, dtype=np.float32)
    cold[:, 128:192] = gate_w
    cold[:, 192:256] = val_w
    cold[:, 256:768] = np.tile(gate_w, NFR)
    cold[:, 768:1280] = np.tile(val_w, NFR)

    in_maps = []
    for c in range(8):
        b, w = c // 2, c % 2
        hot = np.zeros((128, NHOT), np.float32)
        hot[:, 0:128] = query[b].T
        hot[:, 128] = proj_b
        hot[:, 129:193] = q_att_W
        hot[:, 193:257] = qf_att_W
        hot[:, 257:321] = f_att_W
        hot[:, 321] = query_mask[b]
        hot[:, 322] = 1.0
        sw16 = np.zeros((128, 2), np.float16)
        sw16[:, 0] = self_att_W[w].astype(np.float16)
        hot[:, 323] = sw16.view(np.float32)[:, 0]
        rowb = np.zeros((1, NROW), np.float32)
        rowb[0, 0:512] = fragment_mask[b, w].reshape(RL)
        rowb[0, 512:640] = 1.0
        in_maps.append({
            "hot": hot,
            "cold": cold,
            "rowb": rowb,
            "projW": np.ascontiguousarray(proj_W),
            "fragT": np.ascontiguousarray(fragment[b, w].reshape(RL, E).T),
        })
    return in_maps


def kernel(query, fragment, query_mask, fragment_mask, proj_W, proj_b,
           self_att_W, q_att_W, f_att_W, qf_att_W, gate_w, val_w, **_):
    args = [np.asarray(a, np.float32) for a in (
        query, fragment, query_mask, fragment_mask, proj_W, proj_b,
        self_att_W, q_att_W, f_att_W, qf_att_W, gate_w, val_w)]
    query, fragment = args[0], args[1]

    nc = _get_nc()
    in_maps = _make_in_maps(*args)
    _CACHE["last_in_maps"] = in_maps
    res = run_bass_kernel_spmd(nc, in_maps, core_ids=list(range(8))).results

    frag_code = np.zeros((B, NW, NFR, D), np.float32)
    query_code = np.zeros((B, NW, NFR, D), np.float32)
    frag_self_att = np.zeros((B, NW, NFR, LF), np.float32)
    qf_gate = np.zeros((B, NF, LQ), np.float32)
    query_out = np.zeros((B, LQ, D), np.float32)
    for c in range(8):
        b, w = c // 2, c % 2
        r = res[c]
        frag_code[b, w] = np.asarray(r["o_fragcodeT"]).T
        query_code[b, w] = np.asarray(r["o_qcode"])
        frag_self_att[b, w] = np.asarray(r["o_fratt"]).reshape(NFR, LF)
        qf_gate[b, w * NFR:(w + 1) * NFR] = np.asarray(r["o_gate"]).T
        if w == 0:
            query_out[b] = np.asarray(r["o_query"])
    return frag_code, query_code, frag_self_att, qf_gate, query_out


# revision 24
# speedup vs baseline: 1.0802x; 1.0802x over previous
"""Trainium2 Bass kernel for nn_Attention_22342419874323.

Distribution: 8 cores; core c handles batch b = c//2, way w = c%2
(the NFR=8 fragments of one (b, w) pair) -> perfectly data-parallel.

Core algorithm: for each output channel v build a scaled stationary
    A_v[d, q] = qprojT[d, q] * qfW[d, v] + fW[d, v]      (fp16)
so that  matmul(A_v, fragprojT)[q, rl] = qf3[l, q, v] + f_att[l, v]
(the f_att term folds in because sum_d fW[d,v] fragprojT[d,rl] =
f_att[rl,v]).  PSUM holds [q, (r, l)]; the max over l is a free-dim
reduction:
 - "direct" v-groups: DVE tensor_reduce(max) straight from PSUM
 - other groups: ACT copies PSUM -> fp16 SBUF (2 v per op), then DVE
   runs a 2x-packed TT-max tree over 16 v at a time.
gate/val logits then need only two dot products over v, batched on DVE.

Host (numpy) does all transposes / broadcast constants / blob packing;
device does all model FLOPs.
"""

import sys

sys.path.insert(0, "/opt/trn_rl_repo")

import numpy as np

import concourse.bass as bass
import concourse.mybir as mybir
import concourse.tile as tile
from concourse import bacc
from concourse.bass_utils import run_bass_kernel_spmd

B, LQ, LF, NW, NFR, E, D, V = 4, 128, 64, 2, 8, 128, 128, 64
NF = NW * NFR
RL = NFR * LF  # 512
F32 = mybir.dt.float32
F32R = mybir.dt.float32r
F16 = mybir.dt.float16
AF = mybir.ActivationFunctionType
OP = mybir.AluOpType
X = mybir.AxisListType.X

# hot blob (f32 cols): qT 128 | projb 1 | qW 64 | qfW 64 | fW 64 |
#                      qmask 1 | ones_c 1 | selfw16 1
HOT = {"qT": (0, 128), "projb": (128, 1), "qW": (129, 64),
       "qfW": (193, 64), "fW": (257, 64), "qmask": (321, 1),
       "ones_c": (322, 1), "selfw": (323, 1)}
NHOT = 324
# cold blob (f32 cols): ident 128 | gB 64 | vB 64 | gB8 512 | vB8 512
COLD = {"ident": (0, 128), "gB": (128, 64), "vB": (192, 64),
        "gB8": (256, 512), "vB8": (768, 512)}
NCOLD = 1280
# row blob [1, 640]: fmask 512 | ones_r 128
ROW = {"fmask": (0, 512), "ones_r": (512, 128)}
NROW = 640

_CACHE = {}


def _build():
    nc = bacc.Bacc(None, target_bir_lowering=False, debug=False)

    d_hot = nc.dram_tensor("hot", [128, NHOT], F32, kind="ExternalInput")
    d_cold = nc.dram_tensor("cold", [128, NCOLD], F32, kind="ExternalInput")
    d_row = nc.dram_tensor("rowb", [1, NROW], F32, kind="ExternalInput")
    d_projW = nc.dram_tensor("projW", [E, D], F32R, kind="ExternalInput")
    d_fragT = nc.dram_tensor("fragT", [E, RL], F32R, kind="ExternalInput")

    do = {}
    for name, shape in [
        ("o_fragcodeT", [D, NFR]),
        ("o_qcode", [NFR, D]),
        ("o_fratt", [1, RL]),
        ("o_gate", [LQ, NFR]),
        ("o_query", [LQ, D]),
    ]:
        do[name] = nc.dram_tensor(name, shape, F32, kind="ExternalOutput")

    with tile.TileContext(nc) as tc:
        with (
            tc.tile_pool(name="consts", bufs=1) as cpool,
            tc.tile_pool(name="work", bufs=2) as wpool,
            tc.tile_pool(name="av", bufs=6) as apool,
            tc.tile_pool(name="stgp", bufs=3) as stgpool,
            tc.tile_pool(name="psbig", bufs=3, space="PSUM") as psb,
            tc.tile_pool(name="pssm", bufs=2, space="PSUM") as pss,
        ):
            # ---- consolidated input DMAs ----
            t_hot = cpool.tile([128, NHOT], F32, tag="hot")
            t_cold = cpool.tile([128, NCOLD], F32, tag="cold")
            t_row = cpool.tile([1, NROW], F32, tag="row")
            t_projW = cpool.tile([128, D], F32R, tag="projW")
            t_fragT = cpool.tile([128, RL], F32R, tag="fragT")
            nc.sync.dma_start(out=t_hot[:], in_=d_hot[:])
            nc.sync.dma_start(out=t_projW[:], in_=d_projW[:])
            nc.scalar.dma_start(out=t_fragT[:], in_=d_fragT[:])
            nc.scalar.dma_start(out=t_cold[:], in_=d_cold[:])
            nc.sync.dma_start(out=t_row[:], in_=d_row[:])

            def hot(name):
                o, n = HOT[name]
                return t_hot[:, o:o + n]

            def cold(name):
                o, n = COLD[name]
                return t_cold[:, o:o + n]

            def row(name):
                o, n = ROW[name]
                return t_row[:, o:o + n]

            sw16 = hot("selfw").bitcast(F16)[:, 0:1]

            # ---- projections ----
            ps_qpT = pss.tile([128, 128], F32, tag="psq")
            nc.tensor.matmul(ps_qpT[:], lhsT=t_projW[:].bitcast(F32),
                             rhs=hot("qT"), start=True, stop=True)
            sb_qprojT = cpool.tile([128, 128], F32, tag="qprojT")
            nc.vector.tensor_scalar(sb_qprojT[:], ps_qpT[:], hot("projb"),
                                    None, OP.add)
            sb_qprojT16 = cpool.tile([128, 128], F16, tag="qprojT16")
            nc.vector.tensor_copy(sb_qprojT16[:], sb_qprojT[:])

            ps_fpT = psb.tile([128, 2 * RL], F32, tag="psbig")
            nc.tensor.matmul(ps_fpT[:, 0:RL], lhsT=t_projW[:],
                             rhs=t_fragT[:], start=True, stop=True)
            sb_fragprojT = cpool.tile([128, RL], F16, tag="fragprojT")
            nc.vector.tensor_scalar(sb_fragprojT[:], ps_fpT[:, 0:RL],
                                    hot("projb"), None, OP.add)

            # qproj natural [q, d] via PE transpose (+ o_query)
            ps_qn = pss.tile([128, 128], F32, tag="psq")
            nc.tensor.transpose(ps_qn[:], sb_qprojT[:], cold("ident"))
            sb_qprojN = cpool.tile([128, 128], F32, tag="qprojN")
            nc.scalar.copy(sb_qprojN[:], ps_qn[:])
            nc.sync.dma_start(out=do["o_query"][:], in_=sb_qprojN[:])

            # q_att[q, v]
            ps_qa = pss.tile([128, V], F32, tag="psq")
            nc.tensor.matmul(ps_qa[:], lhsT=sb_qprojT[:], rhs=hot("qW"),
                             start=True, stop=True)
            sb_qatt = cpool.tile([128, V], F32, tag="qatt")
            nc.scalar.copy(sb_qatt[:], ps_qa[:])

            # ---- fragment self-attention ----
            ps_fl = pss.tile([1, RL], F32, tag="psq")
            nc.tensor.matmul(ps_fl[:], lhsT=sw16, rhs=sb_fragprojT[:],
                             start=True, stop=True)
            sb_e = wpool.tile([1, RL], F32, tag="fr_e")
            nc.scalar.activation(sb_e[:], ps_fl[:], AF.Exp)
            sb_em = wpool.tile([1, RL], F32, tag="fr_em")
            nc.vector.tensor_tensor(sb_em[:], sb_e[:], row("fmask"), OP.mult)
            sb_sums = wpool.tile([1, NFR], F32, tag="fr_sums")
            nc.vector.tensor_reduce(
                sb_sums[:], sb_em[:].rearrange("p (r l) -> p r l", r=NFR),
                axis=X, op=OP.add)
            sb_rec = wpool.tile([1, NFR], F32, tag="fr_rec")
            nc.vector.tensor_scalar(sb_rec[:], sb_sums[:], 1e-7, None, OP.add)
            sb_rec2 = wpool.tile([1, NFR], F32, tag="fr_rec2")
            nc.vector.reciprocal(sb_rec2[:], sb_rec[:])
            sb_att = wpool.tile([1, RL], F32, tag="fr_att")
            for r in range(NFR):
                nc.vector.tensor_scalar(
                    sb_att[0:1, r * LF:(r + 1) * LF],
                    sb_em[0:1, r * LF:(r + 1) * LF],
                    sb_rec2[0:1, r:r + 1], None, OP.mult)
            nc.sync.dma_start(out=do["o_fratt"][:], in_=sb_att[:])

            # ---- main loop over v ----
            DIRECT = {0, 7}
            sb_M = cpool.tile([128, V * NFR], F32, tag="M")

            def emit_group(g):
                direct = g in DIRECT
                sb_A8 = apool.tile([128, 8 * 128], F16, tag="av")
                eng = nc.vector if direct else nc.gpsimd
                for j in range(8):
                    v = g * 8 + j
                    eng.tensor_scalar(sb_A8[:, j * 128:(j + 1) * 128],
                                      sb_qprojT16[:],
                                      hot("qfW")[:, v:v + 1],
                                      hot("fW")[:, v:v + 1],
                                      OP.mult, OP.add)
                stage_f16 = None
                if not direct:
                    stage_f16 = stgpool.tile([128, 8 * RL], F16,
                                             tag="stg")
                for j2 in range(4):
                    ps_v = psb.tile([128, 2 * RL], F32, tag="psbig")
                    for k in range(2):
                        j = j2 * 2 + k
                        nc.tensor.matmul(
                            ps_v[:, k * RL:(k + 1) * RL],
                            lhsT=sb_A8[:, j * 128:(j + 1) * 128],
                            rhs=sb_fragprojT[:],
                            start=True, stop=True)
                    if direct:
                        nc.vector.tensor_reduce(
                            sb_M[:, (g * 8 + j2 * 2) * NFR:
                                 (g * 8 + j2 * 2 + 2) * NFR],
                            ps_v[:].rearrange("q (w f l) -> q (w f) l",
                                              w=2, f=NFR),
                            axis=X, op=OP.max)
                    else:
                        nc.scalar.copy(
                            stage_f16[:, j2 * 2 * RL:(j2 + 1) * 2 * RL],
                            ps_v[:])
                return stage_f16

            def emit_tree(g0, b0, g1, b1):
                eng = nc.vector
                width = LF
                cur = None
                while width > 2:
                    half = width // 2
                    nxt_t = wpool.tile([128, 128 * half], F16,
                                       tag=f"tr{half}")
                    nxt = nxt_t[:].rearrange("q (g l) -> q g l", g=128)
                    if cur is None:
                        c0 = b0[:].rearrange("q (g l) -> q g l", g=64)
                        c1 = b1[:].rearrange("q (g l) -> q g l", g=64)
                        eng.tensor_tensor(nxt[:, 0:64, :], c0[:, :, 0:half],
                                          c0[:, :, half:width], OP.max)
                        eng.tensor_tensor(nxt[:, 64:128, :], c1[:, :, 0:half],
                                          c1[:, :, half:width], OP.max)
                    else:
                        eng.tensor_tensor(nxt, cur[:, :, 0:half],
                                          cur[:, :, half:width], OP.max)
                    cur = nxt
                    width = half
                eng.tensor_tensor(
                    sb_M[:, g0 * 64:(g0 + 1) * 64]
                    .rearrange("q (g l) -> q g l", g=64),
                    cur[:, 0:64, 0:1], cur[:, 0:64, 1:2], OP.max)
                eng.tensor_tensor(
                    sb_M[:, g1 * 64:(g1 + 1) * 64]
                    .rearrange("q (g l) -> q g l", g=64),
                    cur[:, 64:128, 0:1], cur[:, 64:128, 1:2], OP.max)

            pend = []
            for g in range(V // 8):
                buf = emit_group(g)
                if buf is not None:
                    pend.append((g, buf))
                if len(pend) == 2:
                    (g0, b0), (g1, b1) = pend
                    emit_tree(g0, b0, g1, b1)
                    pend = []

            # ---- gate / val logits ----
            sb_tmp64 = wpool.tile([128, V], F32, tag="tmp64")
            sb_cg = wpool.tile([128, 1], F32, tag="cg")
            sb_cv = wpool.tile([128, 1], F32, tag="cv")
            nc.vector.tensor_tensor(sb_tmp64[:], sb_qatt[:], cold("gB"),
                                    OP.mult)
            nc.vector.tensor_reduce(sb_cg[:], sb_tmp64[:], axis=X, op=OP.add)
            nc.vector.tensor_tensor(sb_tmp64[:], sb_qatt[:], cold("vB"),
                                    OP.mult)
            nc.vector.tensor_reduce(sb_cv[:], sb_tmp64[:], axis=X, op=OP.add)

            M3 = sb_M[:].rearrange("q (v f) -> q f v", v=V)
            gB83 = cold("gB8").rearrange("q (f v) -> q f v", f=NFR)
            vB83 = cold("vB8").rearrange("q (f v) -> q f v", f=NFR)
            sb_prodG = wpool.tile([128, NFR * V], F32, tag="prodG")
            sb_glogr = wpool.tile([128, NFR], F32, tag="glogr")
            sb_glog = wpool.tile([128, NFR], F32, tag="glog")
            nc.vector.tensor_tensor(
                sb_prodG[:].rearrange("q (f v) -> q f v", f=NFR), M3, gB83,
                OP.mult)
            nc.vector.tensor_reduce(
                sb_glogr[:], sb_prodG[:].rearrange("q (f v) -> q f v", f=NFR),
                axis=X, op=OP.add)
            nc.vector.tensor_scalar(sb_glog[:], sb_glogr[:], sb_cg[:, 0:1],
                                    None, OP.add)
            sb_prodV = wpool.tile([128, NFR * V], F32, tag="prodV")
            sb_vlogr = wpool.tile([128, NFR], F32, tag="vlogr")
            sb_vlog = wpool.tile([128, NFR], F32, tag="vlog")
            nc.vector.tensor_tensor(
                sb_prodV[:].rearrange("q (f v) -> q f v", f=NFR), M3, vB83,
                OP.mult)
            nc.vector.tensor_reduce(
                sb_vlogr[:], sb_prodV[:].rearrange("q (f v) -> q f v", f=NFR),
                axis=X, op=OP.add)
            nc.vector.tensor_scalar(sb_vlog[:], sb_vlogr[:], sb_cv[:, 0:1],
                                    None, OP.add)

            # frag_code[d, r] = sum_l fragprojT[d, rl] * att[rl]
            ps_attB = pss.tile([128, RL], F32, tag="psq")
            nc.tensor.matmul(ps_attB[:], lhsT=row("ones_r"), rhs=sb_att[:],
                             start=True, stop=True)
            sb_ab = wpool.tile([128, RL], F32, tag="ab")
            nc.scalar.copy(sb_ab[:], ps_attB[:])
            sb_prod = wpool.tile([128, RL], F32, tag="prod")
            nc.vector.tensor_tensor(sb_prod[:], sb_fragprojT[:], sb_ab[:],
                                    OP.mult)
            sb_fcT = wpool.tile([128, NFR], F32, tag="fcT")
            nc.vector.tensor_reduce(
                sb_fcT[:], sb_prod[:].rearrange("p (r l) -> p r l", r=NFR),
                axis=X, op=OP.add)
            nc.sync.dma_start(out=do["o_fragcodeT"][:], in_=sb_fcT[:])

            # gate = sigmoid(glog) * qmask
            sb_sig = wpool.tile([128, NFR], F32, tag="sig")
            nc.scalar.activation(sb_sig[:], sb_glog[:], AF.Sigmoid)
            sb_gate = wpool.tile([128, NFR], F32, tag="gate")
            nc.vector.tensor_scalar(sb_gate[:], sb_sig[:], hot("qmask"),
                                    None, OP.mult)
            nc.sync.dma_start(out=do["o_gate"][:], in_=sb_gate[:])

            # val softmax over q (partition sum via ones matmul)
            sb_e2 = wpool.tile([128, NFR], F32, tag="e2")
            nc.scalar.activation(sb_e2[:], sb_vlog[:], AF.Exp)
            sb_e2m = wpool.tile([128, NFR], F32, tag="e2m")
            nc.vector.tensor_scalar(sb_e2m[:], sb_e2[:], hot("qmask"),
                                    None, OP.mult)
            ps_s = pss.tile([1, NFR], F32, tag="psq")
            nc.tensor.matmul(ps_s[:], lhsT=hot("ones_c"), rhs=sb_e2m[:],
                             start=True, stop=True)
            sb_s = wpool.tile([1, NFR], F32, tag="s")
            nc.vector.tensor_scalar(sb_s[:], ps_s[:], 1e-7, None, OP.add)
            sb_r2 = wpool.tile([1, NFR], F32, tag="r2")
            nc.vector.reciprocal(sb_r2[:], sb_s[:])
            ps_rb = pss.tile([128, NFR], F32, tag="psq")
            nc.tensor.matmul(ps_rb[:], lhsT=row("ones_r"), rhs=sb_r2[:],
                             start=True, stop=True)
            sb_qfn = wpool.tile([128, NFR], F32, tag="qfn")
            nc.vector.tensor_tensor(sb_qfn[:], sb_e2m[:], ps_rb[:], OP.mult)

            # query_code[f, d] = sum_q qfn[q, f] * qprojN[q, d]
            ps_qc = pss.tile([NFR, 128], F32, tag="psq")
            nc.tensor.matmul(ps_qc[:], lhsT=sb_qfn[:], rhs=sb_qprojN[:],
                             start=True, stop=True)
            sb_qc = wpool.tile([NFR, 128], F32, tag="qc")
            nc.scalar.copy(sb_qc[:], ps_qc[:])
            nc.sync.dma_start(out=do["o_qcode"][:], in_=sb_qc[:])

    nc.compile()
    return nc


def _get_nc():
    if "nc" not in _CACHE:
        _CACHE["nc"] = _build()
    return _CACHE["nc"]


def _make_in_maps(query, fragment, query_mask, fragment_mask, proj_W, proj_b,
                  self_att_W, q_att_W, f_att_W, qf_att_W, gate_w, val_w):
    cold = np.zeros((128, NCOLD), np.float32)
    cold[:, 0:128] = np.eye(128, dtype=np.float32)
    cold[:, 128:192] = gate_w
    cold[:, 192:256] = val_w
    cold[:, 256:768] = np.tile(gate_w, NFR)
    cold[:, 768:1280] = np.tile(val_w, NFR)

    in_maps = []
    for c in range(8):
        b, w = c // 2, c % 2
        hot = np.zeros((128, NHOT), np.float32)
        hot[:, 0:128] = query[b].T
        hot[:, 128] = proj_b
        hot[:, 129:193] = q_att_W
        hot[:, 193:257] = qf_att_W
        hot[:, 257:321] = f_att_W
        hot[:, 321] = query_mask[b]
        hot[:, 322] = 1.0
        sw16 = np.zeros((128, 2), np.float16)
        sw16[:, 0] = self_att_W[w].astype(np.float16)
        hot[:, 323] = sw16.view(np.float32)[:, 0]
        rowb = np.zeros((1, NROW), np.float32)
        rowb[0, 0:512] = fragment_mask[b, w].reshape(RL)
        rowb[0, 512:640] = 1.0
        in_maps.append({
            "hot": hot,
            "cold": cold,
            "rowb": rowb,
            "projW": np.ascontiguousarray(proj_W),
            "fragT": np.ascontiguousarray(fragment[b, w].reshape(RL, E).T),
        })
    return in_maps


def kernel(query, fragment, query_mask, fragment_mask, proj_W, proj_b,
           self_att_W, q_att_W, f_att_W, qf_att_W, gate_w, val_w, **_):
    args = [np.asarray(a, np.float32) for a in (
        query, fragment, query_mask, fragment_mask, proj_W, proj_b,
        self_att_W, q_att_W, f_att_W, qf_att_W, gate_w, val_w)]
    query, fragment = args[0], args[1]

    nc = _get_nc()
    in_maps = _make_in_maps(*args)
    _CACHE["last_in_maps"] = in_maps
    res = run_bass_kernel_spmd(nc, in_maps, core_ids=list(range(8))).results

    frag_code = np.zeros((B, NW, NFR, D), np.float32)
    query_code = np.zeros((B, NW, NFR, D), np.float32)
    frag_self_att = np.zeros((B, NW, NFR, LF), np.float32)
    qf_gate = np.zeros((B, NF, LQ), np.float32)
    query_out = np.zeros((B, LQ, D), np.float32)
    for c in range(8):
        b, w = c // 2, c % 2
        r = res[c]
        frag_code[b, w] = np.asarray(r["o_fragcodeT"]).T
        query_code[b, w] = np.asarray(r["o_qcode"])
        frag_self_att[b, w] = np.asarray(r["o_fratt"]).reshape(NFR, LF)
        qf_gate[b, w * NFR:(w + 1) * NFR] = np.asarray(r["o_gate"]).T
        if w == 0:
            query_out[b] = np.asarray(r["o_query"])
    return frag_code, query_code, frag_self_att, qf_gate, query_out


# revision 25
# speedup vs baseline: 1.1075x; 1.0253x over previous
"""Trainium2 Bass kernel for nn_Attention_22342419874323.

Distribution: 8 cores; core c handles batch b = c//2, way w = c%2
(the NFR=8 fragments of one (b, w) pair) -> perfectly data-parallel.

Core algorithm: for each output channel v build a scaled stationary
    A_v[d, q] = qprojT[d, q] * qfW[d, v] + fW[d, v]      (fp16)
so that  matmul(A_v, fragprojT)[q, rl] = qf3[l, q, v] + f_att[l, v]
(the f_att term folds in because sum_d fW[d,v] fragprojT[d,rl] =
f_att[rl,v]).  PSUM holds [q, (r, l)]; the max over l is a free-dim
reduction:
 - "direct" v-groups: DVE tensor_reduce(max) straight from PSUM
 - other groups: ACT copies PSUM -> fp16 SBUF (2 v per op), then DVE
   runs a 2x-packed TT-max tree over 16 v at a time.
gate/val logits then need only two dot products over v, batched on DVE.

Host (numpy) does all transposes / broadcast constants / blob packing;
device does all model FLOPs.
"""

import sys

sys.path.insert(0, "/opt/trn_rl_repo")

import numpy as np

import concourse.bass as bass
import concourse.mybir as mybir
import concourse.tile as tile
from concourse import bacc
from concourse.bass_utils import run_bass_kernel_spmd

B, LQ, LF, NW, NFR, E, D, V = 4, 128, 64, 2, 8, 128, 128, 64
NF = NW * NFR
RL = NFR * LF  # 512
F32 = mybir.dt.float32
F32R = mybir.dt.float32r
F16 = mybir.dt.float16
AF = mybir.ActivationFunctionType
OP = mybir.AluOpType
X = mybir.AxisListType.X

# hot blob (f32 cols): qT 128 | projb 1 | qW 64 | qfW 64 | fW 64 |
#                      qmask 1 | ones_c 1 | selfw16 1
HOT = {"qT": (0, 128), "projb": (128, 1), "qW": (129, 64),
       "qfW": (193, 64), "fW": (257, 64), "qmask": (321, 1),
       "ones_c": (322, 1), "selfw": (323, 1)}
NHOT = 324
# cold blob (f32 cols): ident 128 | gB 64 | vB 64 | gB8 512 | vB8 512
COLD = {"ident": (0, 128), "gB": (128, 64), "vB": (192, 64),
        "gB8": (256, 512), "vB8": (768, 512)}
NCOLD = 1280
# row blob [1, 640]: fmask 512 | ones_r 128
ROW = {"fmask": (0, 512), "ones_r": (512, 128)}
NROW = 640

_CACHE = {}


def _build():
    nc = bacc.Bacc(None, target_bir_lowering=False, debug=False)

    d_hot = nc.dram_tensor("hot", [128, NHOT], F32, kind="ExternalInput")
    d_cold = nc.dram_tensor("cold", [128, NCOLD], F32, kind="ExternalInput")
    d_row = nc.dram_tensor("rowb", [1, NROW], F32, kind="ExternalInput")
    d_projW = nc.dram_tensor("projW", [E, D], F32R, kind="ExternalInput")
    d_fragT = nc.dram_tensor("fragT", [E, RL], F32R, kind="ExternalInput")

    do = {}
    for name, shape in [
        ("o_fragcodeT", [D, NFR]),
        ("o_qcode", [NFR, D]),
        ("o_fratt", [1, RL]),
        ("o_gate", [LQ, NFR]),
        ("o_query", [LQ, D]),
    ]:
        do[name] = nc.dram_tensor(name, shape, F32, kind="ExternalOutput")

    with tile.TileContext(nc) as tc:
        with (
            tc.tile_pool(name="consts", bufs=1) as cpool,
            tc.tile_pool(name="work", bufs=2) as wpool,
            tc.tile_pool(name="av", bufs=6) as apool,
            tc.tile_pool(name="stgp", bufs=3) as stgpool,
            tc.tile_pool(name="psbig", bufs=3, space="PSUM") as psb,
            tc.tile_pool(name="pssm", bufs=2, space="PSUM") as pss,
        ):
            # ---- consolidated input DMAs ----
            t_hot = cpool.tile([128, NHOT], F32, tag="hot")
            t_cold = cpool.tile([128, NCOLD], F32, tag="cold")
            t_row = cpool.tile([1, NROW], F32, tag="row")
            t_projW = cpool.tile([128, D], F32R, tag="projW")
            t_fragT = cpool.tile([128, RL], F32R, tag="fragT")
            nc.sync.dma_start(out=t_hot[:], in_=d_hot[:])
            nc.sync.dma_start(out=t_projW[:], in_=d_projW[:])
            nc.scalar.dma_start(out=t_fragT[:], in_=d_fragT[:])
            nc.scalar.dma_start(out=t_cold[:], in_=d_cold[:])
            nc.sync.dma_start(out=t_row[:], in_=d_row[:])

            def hot(name):
                o, n = HOT[name]
                return t_hot[:, o:o + n]

            def cold(name):
                o, n = COLD[name]
                return t_cold[:, o:o + n]

            def row(name):
                o, n = ROW[name]
                return t_row[:, o:o + n]

            sw16 = hot("selfw").bitcast(F16)[:, 0:1]

            # ---- PE warmup during input DMA: dummy matmuls on memset
            # tiles to lift HAM to 2.4 GHz before the real stream ----
            wu_a = cpool.tile([128, 128], F16, tag="wu_a")
            wu_b = cpool.tile([128, RL], F16, tag="wu_b")
            nc.vector.memset(wu_a[:], 0.0)
            nc.vector.memset(wu_b[:], 0.0)
            for _ in range(10):
                ps_wu = psb.tile([128, 2 * RL], F32, tag="psbig")
                nc.tensor.matmul(ps_wu[:, 0:RL], lhsT=wu_a[:], rhs=wu_b[:],
                                 start=True, stop=True)

            # ---- projections ----
            ps_qpT = pss.tile([128, 128], F32, tag="psq")
            nc.tensor.matmul(ps_qpT[:], lhsT=t_projW[:].bitcast(F32),
                             rhs=hot("qT"), start=True, stop=True)
            sb_qprojT = cpool.tile([128, 128], F32, tag="qprojT")
            nc.vector.tensor_scalar(sb_qprojT[:], ps_qpT[:], hot("projb"),
                                    None, OP.add)
            sb_qprojT16 = cpool.tile([128, 128], F16, tag="qprojT16")
            nc.vector.tensor_copy(sb_qprojT16[:], sb_qprojT[:])

            ps_fpT = psb.tile([128, 2 * RL], F32, tag="psbig")
            nc.tensor.matmul(ps_fpT[:, 0:RL], lhsT=t_projW[:],
                             rhs=t_fragT[:], start=True, stop=True)
            sb_fragprojT = cpool.tile([128, RL], F16, tag="fragprojT")
            nc.vector.tensor_scalar(sb_fragprojT[:], ps_fpT[:, 0:RL],
                                    hot("projb"), None, OP.add)

            # qproj natural [q, d] via PE transpose (+ o_query)
            ps_qn = pss.tile([128, 128], F32, tag="psq")
            nc.tensor.transpose(ps_qn[:], sb_qprojT[:], cold("ident"))
            sb_qprojN = cpool.tile([128, 128], F32, tag="qprojN")
            nc.scalar.copy(sb_qprojN[:], ps_qn[:])
            nc.sync.dma_start(out=do["o_query"][:], in_=sb_qprojN[:])

            # q_att[q, v]
            ps_qa = pss.tile([128, V], F32, tag="psq")
            nc.tensor.matmul(ps_qa[:], lhsT=sb_qprojT[:], rhs=hot("qW"),
                             start=True, stop=True)
            sb_qatt = cpool.tile([128, V], F32, tag="qatt")
            nc.scalar.copy(sb_qatt[:], ps_qa[:])

            # ---- fragment self-attention ----
            ps_fl = pss.tile([1, RL], F32, tag="psq")
            nc.tensor.matmul(ps_fl[:], lhsT=sw16, rhs=sb_fragprojT[:],
                             start=True, stop=True)
            sb_e = wpool.tile([1, RL], F32, tag="fr_e")
            nc.scalar.activation(sb_e[:], ps_fl[:], AF.Exp)
            sb_em = wpool.tile([1, RL], F32, tag="fr_em")
            nc.vector.tensor_tensor(sb_em[:], sb_e[:], row("fmask"), OP.mult)
            sb_sums = wpool.tile([1, NFR], F32, tag="fr_sums")
            nc.vector.tensor_reduce(
                sb_sums[:], sb_em[:].rearrange("p (r l) -> p r l", r=NFR),
                axis=X, op=OP.add)
            sb_rec = wpool.tile([1, NFR], F32, tag="fr_rec")
            nc.vector.tensor_scalar(sb_rec[:], sb_sums[:], 1e-7, None, OP.add)
            sb_rec2 = wpool.tile([1, NFR], F32, tag="fr_rec2")
            nc.vector.reciprocal(sb_rec2[:], sb_rec[:])
            sb_att = wpool.tile([1, RL], F32, tag="fr_att")
            for r in range(NFR):
                nc.vector.tensor_scalar(
                    sb_att[0:1, r * LF:(r + 1) * LF],
                    sb_em[0:1, r * LF:(r + 1) * LF],
                    sb_rec2[0:1, r:r + 1], None, OP.mult)
            nc.sync.dma_start(out=do["o_fratt"][:], in_=sb_att[:])

            # ---- main loop over v ----
            DIRECT = {0, 7}
            sb_M = cpool.tile([128, V * NFR], F32, tag="M")

            def emit_group(g):
                direct = g in DIRECT
                sb_A8 = apool.tile([128, 8 * 128], F16, tag="av")
                eng = nc.vector if direct else nc.gpsimd
                for j in range(8):
                    v = g * 8 + j
                    eng.tensor_scalar(sb_A8[:, j * 128:(j + 1) * 128],
                                      sb_qprojT16[:],
                                      hot("qfW")[:, v:v + 1],
                                      hot("fW")[:, v:v + 1],
                                      OP.mult, OP.add)
                stage_f16 = None
                if not direct:
                    stage_f16 = stgpool.tile([128, 8 * RL], F16,
                                             tag="stg")
                for j2 in range(4):
                    ps_v = psb.tile([128, 2 * RL], F32, tag="psbig")
                    for k in range(2):
                        j = j2 * 2 + k
                        nc.tensor.matmul(
                            ps_v[:, k * RL:(k + 1) * RL],
                            lhsT=sb_A8[:, j * 128:(j + 1) * 128],
                            rhs=sb_fragprojT[:],
                            start=True, stop=True)
                    if direct:
                        nc.vector.tensor_reduce(
                            sb_M[:, (g * 8 + j2 * 2) * NFR:
                                 (g * 8 + j2 * 2 + 2) * NFR],
                            ps_v[:].rearrange("q (w f l) -> q (w f) l",
                                              w=2, f=NFR),
                            axis=X, op=OP.max)
                    else:
                        nc.scalar.copy(
                            stage_f16[:, j2 * 2 * RL:(j2 + 1) * 2 * RL],
                            ps_v[:])
                return stage_f16

            def emit_tree(g0, b0, g1, b1):
                eng = nc.vector
                width = LF
                cur = None
                while width > 2:
                    half = width // 2
                    nxt_t = wpool.tile([128, 128 * half], F16,
                                       tag=f"tr{half}")
                    nxt = nxt_t[:].rearrange("q (g l) -> q g l", g=128)
                    if cur is None:
                        c0 = b0[:].rearrange("q (g l) -> q g l", g=64)
                        c1 = b1[:].rearrange("q (g l) -> q g l", g=64)
                        eng.tensor_tensor(nxt[:, 0:64, :], c0[:, :, 0:half],
                                          c0[:, :, half:width], OP.max)
                        eng.tensor_tensor(nxt[:, 64:128, :], c1[:, :, 0:half],
                                          c1[:, :, half:width], OP.max)
                    else:
                        eng.tensor_tensor(nxt, cur[:, :, 0:half],
                                          cur[:, :, half:width], OP.max)
                    cur = nxt
                    width = half
                eng.tensor_tensor(
                    sb_M[:, g0 * 64:(g0 + 1) * 64]
                    .rearrange("q (g l) -> q g l", g=64),
                    cur[:, 0:64, 0:1], cur[:, 0:64, 1:2], OP.max)
                eng.tensor_tensor(
                    sb_M[:, g1 * 64:(g1 + 1) * 64]
                    .rearrange("q (g l) -> q g l", g=64),
                    cur[:, 64:128, 0:1], cur[:, 64:128, 1:2], OP.max)

            pend = []
            for g in range(V // 8):
                buf = emit_group(g)
                if buf is not None:
                    pend.append((g, buf))
                if len(pend) == 2:
                    (g0, b0), (g1, b1) = pend
                    emit_tree(g0, b0, g1, b1)
                    pend = []

            # ---- gate / val logits ----
            sb_tmp64 = wpool.tile([128, V], F32, tag="tmp64")
            sb_cg = wpool.tile([128, 1], F32, tag="cg")
            sb_cv = wpool.tile([128, 1], F32, tag="cv")
            nc.vector.tensor_tensor(sb_tmp64[:], sb_qatt[:], cold("gB"),
                                    OP.mult)
            nc.vector.tensor_reduce(sb_cg[:], sb_tmp64[:], axis=X, op=OP.add)
            nc.vector.tensor_tensor(sb_tmp64[:], sb_qatt[:], cold("vB"),
                                    OP.mult)
            nc.vector.tensor_reduce(sb_cv[:], sb_tmp64[:], axis=X, op=OP.add)

            M3 = sb_M[:].rearrange("q (v f) -> q f v", v=V)
            gB83 = cold("gB8").rearrange("q (f v) -> q f v", f=NFR)
            vB83 = cold("vB8").rearrange("q (f v) -> q f v", f=NFR)
            sb_prodG = wpool.tile([128, NFR * V], F32, tag="prodG")
            sb_glogr = wpool.tile([128, NFR], F32, tag="glogr")
            sb_glog = wpool.tile([128, NFR], F32, tag="glog")
            nc.vector.tensor_tensor(
                sb_prodG[:].rearrange("q (f v) -> q f v", f=NFR), M3, gB83,
                OP.mult)
            nc.vector.tensor_reduce(
                sb_glogr[:], sb_prodG[:].rearrange("q (f v) -> q f v", f=NFR),
                axis=X, op=OP.add)
            nc.vector.tensor_scalar(sb_glog[:], sb_glogr[:], sb_cg[:, 0:1],
                                    None, OP.add)
            sb_prodV = wpool.tile([128, NFR * V], F32, tag="prodV")
            sb_vlogr = wpool.tile([128, NFR], F32, tag="vlogr")
            sb_vlog = wpool.tile([128, NFR], F32, tag="vlog")
            nc.vector.tensor_tensor(
                sb_prodV[:].rearrange("q (f v) -> q f v", f=NFR), M3, vB83,
                OP.mult)
            nc.vector.tensor_reduce(
                sb_vlogr[:], sb_prodV[:].rearrange("q (f v) -> q f v", f=NFR),
                axis=X, op=OP.add)
            nc.vector.tensor_scalar(sb_vlog[:], sb_vlogr[:], sb_cv[:, 0:1],
                                    None, OP.add)

            # frag_code[d, r] = sum_l fragprojT[d, rl] * att[rl]
            ps_attB = pss.tile([128, RL], F32, tag="psq")
            nc.tensor.matmul(ps_attB[:], lhsT=row("ones_r"), rhs=sb_att[:],
                             start=True, stop=True)
            sb_ab = wpool.tile([128, RL], F32, tag="ab")
            nc.scalar.copy(sb_ab[:], ps_attB[:])
            sb_prod = wpool.tile([128, RL], F32, tag="prod")
            nc.vector.tensor_tensor(sb_prod[:], sb_fragprojT[:], sb_ab[:],
                                    OP.mult)
            sb_fcT = wpool.tile([128, NFR], F32, tag="fcT")
            nc.vector.tensor_reduce(
                sb_fcT[:], sb_prod[:].rearrange("p (r l) -> p r l", r=NFR),
                axis=X, op=OP.add)
            nc.sync.dma_start(out=do["o_fragcodeT"][:], in_=sb_fcT[:])

            # gate = sigmoid(glog) * qmask
            sb_sig = wpool.tile([128, NFR], F32, tag="sig")
            nc.scalar.activation(sb_sig[:], sb_glog[:], AF.Sigmoid)
            sb_gate = wpool.tile([128, NFR], F32, tag="gate")
            nc.vector.tensor_scalar(sb_gate[:], sb_sig[:], hot("qmask"),
                                    None, OP.mult)
            nc.sync.dma_start(out=do["o_gate"][:], in_=sb_gate[:])

            # val softmax over q (partition sum via ones matmul)
            sb_e2 = wpool.tile([128, NFR], F32, tag="e2")
            nc.scalar.activation(sb_e2[:], sb_vlog[:], AF.Exp)
            sb_e2m = wpool.tile([128, NFR], F32, tag="e2m")
            nc.vector.tensor_scalar(sb_e2m[:], sb_e2[:], hot("qmask"),
                                    None, OP.mult)
            ps_s = pss.tile([1, NFR], F32, tag="psq")
            nc.tensor.matmul(ps_s[:], lhsT=hot("ones_c"), rhs=sb_e2m[:],
                             start=True, stop=True)
            sb_s = wpool.tile([1, NFR], F32, tag="s")
            nc.vector.tensor_scalar(sb_s[:], ps_s[:], 1e-7, None, OP.add)
            sb_r2 = wpool.tile([1, NFR], F32, tag="r2")
            nc.vector.reciprocal(sb_r2[:], sb_s[:])
            ps_rb = pss.tile([128, NFR], F32, tag="psq")
            nc.tensor.matmul(ps_rb[:], lhsT=row("ones_r"), rhs=sb_r2[:],
                             start=True, stop=True)
            sb_qfn = wpool.tile([128, NFR], F32, tag="qfn")
            nc.vector.tensor_tensor(sb_qfn[:], sb_e2m[:], ps_rb[:], OP.mult)

            # query_code[f, d] = sum_q qfn[q, f] * qprojN[q, d]
            ps_qc = pss.tile([NFR, 128], F32, tag="psq")
            nc.tensor.matmul(ps_qc[:], lhsT=sb_qfn[:], rhs=sb_qprojN[:],
                             start=True, stop=True)
            sb_qc = wpool.tile([NFR, 128], F32, tag="qc")
            nc.scalar.copy(sb_qc[:], ps_qc[:])
            nc.sync.dma_start(out=do["o_qcode"][:], in_=sb_qc[:])

    nc.compile()
    return nc


def _get_nc():
    if "nc" not in _CACHE:
        _CACHE["nc"] = _build()
    return _CACHE["nc"]


def _make_in_maps(query, fragment, query_mask, fragment_mask, proj_W, proj_b,
                  self_att_W, q_att_W, f_att_W, qf_att_W, gate_w, val_w):
    cold = np.zeros((128, NCOLD), np.float32)
    cold[:, 0:128] = np.eye(128, dtype=np.float32)
    cold[:, 128:192] = gate_w
    cold[:, 192:256] = val_w
    cold[:, 256:768] = np.tile(gate_w, NFR)
    cold[:, 768:1280] = np.tile(val_w, NFR)

    in_maps = []
    for c in range(8):
        b, w = c // 2, c % 2
        hot = np.zeros((128, NHOT), np.float32)
        hot[:, 0:128] = query[b].T
        hot[:, 128] = proj_b
        hot[:, 129:193] = q_att_W
        hot[:, 193:257] = qf_att_W
        hot[:, 257:321] = f_att_W
        hot[:, 321] = query_mask[b]
        hot[:, 322] = 1.0
        sw16 = np.zeros((128, 2), np.float16)
        sw16[:, 0] = self_att_W[w].astype(np.float16)
        hot[:, 323] = sw16.view(np.float32)[:, 0]
        rowb = np.zeros((1, NROW), np.float32)
        rowb[0, 0:512] = fragment_mask[b, w].reshape(RL)
        rowb[0, 512:640] = 1.0
        in_maps.append({
            "hot": hot,
            "cold": cold,
            "rowb": rowb,
            "projW": np.ascontiguousarray(proj_W),
            "fragT": np.ascontiguousarray(fragment[b, w].reshape(RL, E).T),
        })
    return in_maps


def kernel(query, fragment, query_mask, fragment_mask, proj_W, proj_b,
           self_att_W, q_att_W, f_att_W, qf_att_W, gate_w, val_w, **_):
    args = [np.asarray(a, np.float32) for a in (
        query, fragment, query_mask, fragment_mask, proj_W, proj_b,
        self_att_W, q_att_W, f_att_W, qf_att_W, gate_w, val_w)]
    query, fragment = args[0], args[1]

    nc = _get_nc()
    in_maps = _make_in_maps(*args)
    _CACHE["last_in_maps"] = in_maps
    res = run_bass_kernel_spmd(nc, in_maps, core_ids=list(range(8))).results

    frag_code = np.zeros((B, NW, NFR, D), np.float32)
    query_code = np.zeros((B, NW, NFR, D), np.float32)
    frag_self_att = np.zeros((B, NW, NFR, LF), np.float32)
    qf_gate = np.zeros((B, NF, LQ), np.float32)
    query_out = np.zeros((B, LQ, D), np.float32)
    for c in range(8):
        b, w = c // 2, c % 2
        r = res[c]
        frag_code[b, w] = np.asarray(r["o_fragcodeT"]).T
        query_code[b, w] = np.asarray(r["o_qcode"])
        frag_self_att[b, w] = np.asarray(r["o_fratt"]).reshape(NFR, LF)
        qf_gate[b, w * NFR:(w + 1) * NFR] = np.asarray(r["o_gate"]).T
        if w == 0:
            query_out[b] = np.asarray(r["o_query"])
    return frag_code, query_code, frag_self_att, qf_gate, query_out


# revision 26
# speedup vs baseline: 1.1347x; 1.0246x over previous
"""Trainium2 Bass kernel for nn_Attention_22342419874323.

Distribution: 8 cores; core c handles batch b = c//2, way w = c%2
(the NFR=8 fragments of one (b, w) pair) -> perfectly data-parallel.

Core algorithm: for each output channel v build a scaled stationary
    A_v[d, q] = qprojT[d, q] * qfW[d, v] + fW[d, v]      (fp16)
so that  matmul(A_v, fragprojT)[q, rl] = qf3[l, q, v] + f_att[l, v]
(the f_att term folds in because sum_d fW[d,v] fragprojT[d,rl] =
f_att[rl,v]).  PSUM holds [q, (r, l)]; the max over l is a free-dim
reduction:
 - "direct" v-groups: DVE tensor_reduce(max) straight from PSUM
 - other groups: ACT copies PSUM -> fp16 SBUF (2 v per op), then DVE
   runs a 2x-packed TT-max tree over 16 v at a time.
gate/val logits then need only two dot products over v, batched on DVE.

Host (numpy) does all transposes / broadcast constants / blob packing;
device does all model FLOPs.
"""

import sys

sys.path.insert(0, "/opt/trn_rl_repo")

import numpy as np

import concourse.bass as bass
import concourse.mybir as mybir
import concourse.tile as tile
from concourse import bacc
from concourse.bass_utils import run_bass_kernel_spmd

B, LQ, LF, NW, NFR, E, D, V = 4, 128, 64, 2, 8, 128, 128, 64
NF = NW * NFR
RL = NFR * LF  # 512
F32 = mybir.dt.float32
F32R = mybir.dt.float32r
F16 = mybir.dt.float16
AF = mybir.ActivationFunctionType
OP = mybir.AluOpType
X = mybir.AxisListType.X

# hot blob (f32 cols): qT 128 | projb 1 | qW 64 | qfW 64 | fW 64 |
#                      qmask 1 | ones_c 1 | selfw16 1
HOT = {"qT": (0, 128), "projb": (128, 1), "qW": (129, 64),
       "qfW": (193, 64), "fW": (257, 64), "qmask": (321, 1),
       "ones_c": (322, 1), "selfw": (323, 1)}
NHOT = 324
# cold blob (f32 cols): ident 128 | gB 64 | vB 64 | gB8 512 | vB8 512
COLD = {"ident": (0, 128), "gB": (128, 64), "vB": (192, 64),
        "gB8": (256, 512), "vB8": (768, 512)}
NCOLD = 1280
# row blob [1, 640]: fmask 512 | ones_r 128
ROW = {"fmask": (0, 512), "ones_r": (512, 128)}
NROW = 640

_CACHE = {}


def _build():
    nc = bacc.Bacc(None, target_bir_lowering=False, debug=False)

    d_hot = nc.dram_tensor("hot", [128, NHOT], F32, kind="ExternalInput")
    d_cold = nc.dram_tensor("cold", [128, NCOLD], F32, kind="ExternalInput")
    d_row = nc.dram_tensor("rowb", [1, NROW], F32, kind="ExternalInput")
    d_projW = nc.dram_tensor("projW", [E, D], F32R, kind="ExternalInput")
    d_fragT = nc.dram_tensor("fragT", [E, RL], F32R, kind="ExternalInput")

    do = {}
    for name, shape in [
        ("o_fragcodeT", [D, NFR]),
        ("o_qcode", [NFR, D]),
        ("o_fratt", [1, RL]),
        ("o_gate", [LQ, NFR]),
        ("o_query", [LQ, D]),
    ]:
        do[name] = nc.dram_tensor(name, shape, F32, kind="ExternalOutput")

    with tile.TileContext(nc) as tc:
        with (
            tc.tile_pool(name="consts", bufs=1) as cpool,
            tc.tile_pool(name="work", bufs=2) as wpool,
            tc.tile_pool(name="av", bufs=8) as apool,
            tc.tile_pool(name="stgp", bufs=4) as stgpool,
            tc.tile_pool(name="psbig", bufs=3, space="PSUM") as psb,
            tc.tile_pool(name="pssm", bufs=2, space="PSUM") as pss,
        ):
            # ---- consolidated input DMAs ----
            t_hot = cpool.tile([128, NHOT], F32, tag="hot")
            t_cold = cpool.tile([128, NCOLD], F32, tag="cold")
            t_row = cpool.tile([1, NROW], F32, tag="row")
            t_projW = cpool.tile([128, D], F32R, tag="projW")
            t_fragT = cpool.tile([128, RL], F32R, tag="fragT")
            nc.sync.dma_start(out=t_hot[:], in_=d_hot[:])
            nc.sync.dma_start(out=t_projW[:], in_=d_projW[:])
            nc.scalar.dma_start(out=t_fragT[:], in_=d_fragT[:])
            nc.scalar.dma_start(out=t_cold[:], in_=d_cold[:])
            nc.sync.dma_start(out=t_row[:], in_=d_row[:])

            def hot(name):
                o, n = HOT[name]
                return t_hot[:, o:o + n]

            def cold(name):
                o, n = COLD[name]
                return t_cold[:, o:o + n]

            def row(name):
                o, n = ROW[name]
                return t_row[:, o:o + n]

            sw16 = hot("selfw").bitcast(F16)[:, 0:1]

            # ---- PE warmup during input DMA: dummy matmuls on memset
            # tiles to lift HAM to 2.4 GHz before the real stream ----
            wu_a = cpool.tile([128, 128], F16, tag="wu_a")
            wu_b = cpool.tile([128, RL], F16, tag="wu_b")
            nc.vector.memset(wu_a[:], 0.0)
            nc.vector.memset(wu_b[:], 0.0)
            for _ in range(10):
                ps_wu = psb.tile([128, 2 * RL], F32, tag="psbig")
                nc.tensor.matmul(ps_wu[:, 0:RL], lhsT=wu_a[:], rhs=wu_b[:],
                                 start=True, stop=True)

            # ---- projections ----
            ps_qpT = pss.tile([128, 128], F32, tag="psq")
            nc.tensor.matmul(ps_qpT[:], lhsT=t_projW[:].bitcast(F32),
                             rhs=hot("qT"), start=True, stop=True)
            sb_qprojT = cpool.tile([128, 128], F32, tag="qprojT")
            nc.vector.tensor_scalar(sb_qprojT[:], ps_qpT[:], hot("projb"),
                                    None, OP.add)
            sb_qprojT16 = cpool.tile([128, 128], F16, tag="qprojT16")
            nc.vector.tensor_copy(sb_qprojT16[:], sb_qprojT[:])

            ps_fpT = psb.tile([128, 2 * RL], F32, tag="psbig")
            nc.tensor.matmul(ps_fpT[:, 0:RL], lhsT=t_projW[:],
                             rhs=t_fragT[:], start=True, stop=True)
            sb_fragprojT = cpool.tile([128, RL], F16, tag="fragprojT")
            nc.vector.tensor_scalar(sb_fragprojT[:], ps_fpT[:, 0:RL],
                                    hot("projb"), None, OP.add)

            # qproj natural [q, d] via PE transpose (+ o_query)
            ps_qn = pss.tile([128, 128], F32, tag="psq")
            nc.tensor.transpose(ps_qn[:], sb_qprojT[:], cold("ident"))
            sb_qprojN = cpool.tile([128, 128], F32, tag="qprojN")
            nc.scalar.copy(sb_qprojN[:], ps_qn[:])
            nc.sync.dma_start(out=do["o_query"][:], in_=sb_qprojN[:])

            # q_att[q, v]
            ps_qa = pss.tile([128, V], F32, tag="psq")
            nc.tensor.matmul(ps_qa[:], lhsT=sb_qprojT[:], rhs=hot("qW"),
                             start=True, stop=True)
            sb_qatt = cpool.tile([128, V], F32, tag="qatt")
            nc.scalar.copy(sb_qatt[:], ps_qa[:])

            # ---- fragment self-attention ----
            ps_fl = pss.tile([1, RL], F32, tag="psq")
            nc.tensor.matmul(ps_fl[:], lhsT=sw16, rhs=sb_fragprojT[:],
                             start=True, stop=True)
            sb_e = wpool.tile([1, RL], F32, tag="fr_e")
            nc.scalar.activation(sb_e[:], ps_fl[:], AF.Exp)
            sb_em = wpool.tile([1, RL], F32, tag="fr_em")
            nc.vector.tensor_tensor(sb_em[:], sb_e[:], row("fmask"), OP.mult)
            sb_sums = wpool.tile([1, NFR], F32, tag="fr_sums")
            nc.vector.tensor_reduce(
                sb_sums[:], sb_em[:].rearrange("p (r l) -> p r l", r=NFR),
                axis=X, op=OP.add)
            sb_rec = wpool.tile([1, NFR], F32, tag="fr_rec")
            nc.vector.tensor_scalar(sb_rec[:], sb_sums[:], 1e-7, None, OP.add)
            sb_rec2 = wpool.tile([1, NFR], F32, tag="fr_rec2")
            nc.vector.reciprocal(sb_rec2[:], sb_rec[:])
            sb_att = wpool.tile([1, RL], F32, tag="fr_att")
            for r in range(NFR):
                nc.vector.tensor_scalar(
                    sb_att[0:1, r * LF:(r + 1) * LF],
                    sb_em[0:1, r * LF:(r + 1) * LF],
                    sb_rec2[0:1, r:r + 1], None, OP.mult)
            nc.sync.dma_start(out=do["o_fratt"][:], in_=sb_att[:])

            # ---- main loop over v ----
            DIRECT = {0, 7}
            sb_M = cpool.tile([128, V * NFR], F32, tag="M")

            def emit_group(g):
                direct = g in DIRECT
                sb_A8 = apool.tile([128, 8 * 128], F16, tag="av")
                eng = nc.vector if direct else nc.gpsimd
                for j in range(8):
                    v = g * 8 + j
                    eng.tensor_scalar(sb_A8[:, j * 128:(j + 1) * 128],
                                      sb_qprojT16[:],
                                      hot("qfW")[:, v:v + 1],
                                      hot("fW")[:, v:v + 1],
                                      OP.mult, OP.add)
                stage_f16 = None
                if not direct:
                    stage_f16 = stgpool.tile([128, 8 * RL], F16,
                                             tag="stg")
                for j2 in range(4):
                    ps_v = psb.tile([128, 2 * RL], F32, tag="psbig")
                    for k in range(2):
                        j = j2 * 2 + k
                        nc.tensor.matmul(
                            ps_v[:, k * RL:(k + 1) * RL],
                            lhsT=sb_A8[:, j * 128:(j + 1) * 128],
                            rhs=sb_fragprojT[:],
                            start=True, stop=True)
                    if direct:
                        nc.vector.tensor_reduce(
                            sb_M[:, (g * 8 + j2 * 2) * NFR:
                                 (g * 8 + j2 * 2 + 2) * NFR],
                            ps_v[:].rearrange("q (w f l) -> q (w f) l",
                                              w=2, f=NFR),
                            axis=X, op=OP.max)
                    else:
                        nc.scalar.copy(
                            stage_f16[:, j2 * 2 * RL:(j2 + 1) * 2 * RL],
                            ps_v[:])
                return stage_f16

            def emit_tree(g0, b0, g1, b1):
                eng = nc.vector
                width = LF
                cur = None
                while width > 2:
                    half = width // 2
                    nxt_t = wpool.tile([128, 128 * half], F16,
                                       tag=f"tr{half}")
                    nxt = nxt_t[:].rearrange("q (g l) -> q g l", g=128)
                    if cur is None:
                        c0 = b0[:].rearrange("q (g l) -> q g l", g=64)
                        c1 = b1[:].rearrange("q (g l) -> q g l", g=64)
                        eng.tensor_tensor(nxt[:, 0:64, :], c0[:, :, 0:half],
                                          c0[:, :, half:width], OP.max)
                        eng.tensor_tensor(nxt[:, 64:128, :], c1[:, :, 0:half],
                                          c1[:, :, half:width], OP.max)
                    else:
                        eng.tensor_tensor(nxt, cur[:, :, 0:half],
                                          cur[:, :, half:width], OP.max)
                    cur = nxt
                    width = half
                eng.tensor_tensor(
                    sb_M[:, g0 * 64:(g0 + 1) * 64]
                    .rearrange("q (g l) -> q g l", g=64),
                    cur[:, 0:64, 0:1], cur[:, 0:64, 1:2], OP.max)
                eng.tensor_tensor(
                    sb_M[:, g1 * 64:(g1 + 1) * 64]
                    .rearrange("q (g l) -> q g l", g=64),
                    cur[:, 64:128, 0:1], cur[:, 64:128, 1:2], OP.max)

            pend = []
            for g in range(V // 8):
                buf = emit_group(g)
                if buf is not None:
                    pend.append((g, buf))
                if len(pend) == 2:
                    (g0, b0), (g1, b1) = pend
                    emit_tree(g0, b0, g1, b1)
                    pend = []

            # ---- gate / val logits ----
            sb_tmp64 = wpool.tile([128, V], F32, tag="tmp64")
            sb_cg = wpool.tile([128, 1], F32, tag="cg")
            sb_cv = wpool.tile([128, 1], F32, tag="cv")
            nc.vector.tensor_tensor(sb_tmp64[:], sb_qatt[:], cold("gB"),
                                    OP.mult)
            nc.vector.tensor_reduce(sb_cg[:], sb_tmp64[:], axis=X, op=OP.add)
            nc.vector.tensor_tensor(sb_tmp64[:], sb_qatt[:], cold("vB"),
                                    OP.mult)
            nc.vector.tensor_reduce(sb_cv[:], sb_tmp64[:], axis=X, op=OP.add)

            M3 = sb_M[:].rearrange("q (v f) -> q f v", v=V)
            gB83 = cold("gB8").rearrange("q (f v) -> q f v", f=NFR)
            vB83 = cold("vB8").rearrange("q (f v) -> q f v", f=NFR)
            sb_prodG = wpool.tile([128, NFR * V], F32, tag="prodG")
            sb_glogr = wpool.tile([128, NFR], F32, tag="glogr")
            sb_glog = wpool.tile([128, NFR], F32, tag="glog")
            nc.vector.tensor_tensor(
                sb_prodG[:].rearrange("q (f v) -> q f v", f=NFR), M3, gB83,
                OP.mult)
            nc.vector.tensor_reduce(
                sb_glogr[:], sb_prodG[:].rearrange("q (f v) -> q f v", f=NFR),
                axis=X, op=OP.add)
            nc.vector.tensor_scalar(sb_glog[:], sb_glogr[:], sb_cg[:, 0:1],
                                    None, OP.add)
            sb_prodV = wpool.tile([128, NFR * V], F32, tag="prodV")
            sb_vlogr = wpool.tile([128, NFR], F32, tag="vlogr")
            sb_vlog = wpool.tile([128, NFR], F32, tag="vlog")
            nc.vector.tensor_tensor(
                sb_prodV[:].rearrange("q (f v) -> q f v", f=NFR), M3, vB83,
                OP.mult)
            nc.vector.tensor_reduce(
                sb_vlogr[:], sb_prodV[:].rearrange("q (f v) -> q f v", f=NFR),
                axis=X, op=OP.add)
            nc.vector.tensor_scalar(sb_vlog[:], sb_vlogr[:], sb_cv[:, 0:1],
                                    None, OP.add)

            # frag_code[d, r] = sum_l fragprojT[d, rl] * att[rl]
            ps_attB = pss.tile([128, RL], F32, tag="psq")
            nc.tensor.matmul(ps_attB[:], lhsT=row("ones_r"), rhs=sb_att[:],
                             start=True, stop=True)
            sb_ab = wpool.tile([128, RL], F32, tag="ab")
            nc.scalar.copy(sb_ab[:], ps_attB[:])
            sb_prod = wpool.tile([128, RL], F32, tag="prod")
            nc.vector.tensor_tensor(sb_prod[:], sb_fragprojT[:], sb_ab[:],
                                    OP.mult)
            sb_fcT = wpool.tile([128, NFR], F32, tag="fcT")
            nc.vector.tensor_reduce(
                sb_fcT[:], sb_prod[:].rearrange("p (r l) -> p r l", r=NFR),
                axis=X, op=OP.add)
            nc.sync.dma_start(out=do["o_fragcodeT"][:], in_=sb_fcT[:])

            # gate = sigmoid(glog) * qmask
            sb_sig = wpool.tile([128, NFR], F32, tag="sig")
            nc.scalar.activation(sb_sig[:], sb_glog[:], AF.Sigmoid)
            sb_gate = wpool.tile([128, NFR], F32, tag="gate")
            nc.vector.tensor_scalar(sb_gate[:], sb_sig[:], hot("qmask"),
                                    None, OP.mult)
            nc.sync.dma_start(out=do["o_gate"][:], in_=sb_gate[:])

            # val softmax over q (partition sum via ones matmul)
            sb_e2 = wpool.tile([128, NFR], F32, tag="e2")
            nc.scalar.activation(sb_e2[:], sb_vlog[:], AF.Exp)
            sb_e2m = wpool.tile([128, NFR], F32, tag="e2m")
            nc.vector.tensor_scalar(sb_e2m[:], sb_e2[:], hot("qmask"),
                                    None, OP.mult)
            ps_s = pss.tile([1, NFR], F32, tag="psq")
            nc.tensor.matmul(ps_s[:], lhsT=hot("ones_c"), rhs=sb_e2m[:],
                             start=True, stop=True)
            sb_s = wpool.tile([1, NFR], F32, tag="s")
            nc.vector.tensor_scalar(sb_s[:], ps_s[:], 1e-7, None, OP.add)
            sb_r2 = wpool.tile([1, NFR], F32, tag="r2")
            nc.vector.reciprocal(sb_r2[:], sb_s[:])
            ps_rb = pss.tile([128, NFR], F32, tag="psq")
            nc.tensor.matmul(ps_rb[:], lhsT=row("ones_r"), rhs=sb_r2[:],
                             start=True, stop=True)
            sb_qfn = wpool.tile([128, NFR], F32, tag="qfn")
            nc.vector.tensor_tensor(sb_qfn[:], sb_e2m[:], ps_rb[:], OP.mult)

            # query_code[f, d] = sum_q qfn[q, f] * qprojN[q, d]
            ps_qc = pss.tile([NFR, 128], F32, tag="psq")
            nc.tensor.matmul(ps_qc[:], lhsT=sb_qfn[:], rhs=sb_qprojN[:],
                             start=True, stop=True)
            sb_qc = wpool.tile([NFR, 128], F32, tag="qc")
            nc.scalar.copy(sb_qc[:], ps_qc[:])
            nc.sync.dma_start(out=do["o_qcode"][:], in_=sb_qc[:])

    nc.compile()
    return nc


def _get_nc():
    if "nc" not in _CACHE:
        _CACHE["nc"] = _build()
    return _CACHE["nc"]


def _make_in_maps(query, fragment, query_mask, fragment_mask, proj_W, proj_b,
                  self_att_W, q_att_W, f_att_W, qf_att_W, gate_w, val_w):
    cold = np.zeros((128, NCOLD), np.float32)
    cold[:, 0:128] = np.eye(128, dtype=np.float32)
    cold[:, 128:192] = gate_w
    cold[:, 192:256] = val_w
    cold[:, 256:768] = np.tile(gate_w, NFR)
    cold[:, 768:1280] = np.tile(val_w, NFR)

    in_maps = []
    for c in range(8):
        b, w = c // 2, c % 2
        hot = np.zeros((128, NHOT), np.float32)
        hot[:, 0:128] = query[b].T
        hot[:, 128] = proj_b
        hot[:, 129:193] = q_att_W
        hot[:, 193:257] = qf_att_W
        hot[:, 257:321] = f_att_W
        hot[:, 321] = query_mask[b]
        hot[:, 322] = 1.0
        sw16 = np.zeros((128, 2), np.float16)
        sw16[:, 0] = self_att_W[w].astype(np.float16)
        hot[:, 323] = sw16.view(np.float32)[:, 0]
        rowb = np.zeros((1, NROW), np.float32)
        rowb[0, 0:512] = fragment_mask[b, w].reshape(RL)
        rowb[0, 512:640] = 1.0
        in_maps.append({
            "hot": hot,
            "cold": cold,
            "rowb": rowb,
            "projW": np.ascontiguousarray(proj_W),
            "fragT": np.ascontiguousarray(fragment[b, w].reshape(RL, E).T),
        })
    return in_maps


def kernel(query, fragment, query_mask, fragment_mask, proj_W, proj_b,
           self_att_W, q_att_W, f_att_W, qf_att_W, gate_w, val_w, **_):
    args = [np.asarray(a, np.float32) for a in (
        query, fragment, query_mask, fragment_mask, proj_W, proj_b,
        self_att_W, q_att_W, f_att_W, qf_att_W, gate_w, val_w)]
    query, fragment = args[0], args[1]

    nc = _get_nc()
    in_maps = _make_in_maps(*args)
    _CACHE["last_in_maps"] = in_maps
    res = run_bass_kernel_spmd(nc, in_maps, core_ids=list(range(8))).results

    frag_code = np.zeros((B, NW, NFR, D), np.float32)
    query_code = np.zeros((B, NW, NFR, D), np.float32)
    frag_self_att = np.zeros((B, NW, NFR, LF), np.float32)
    qf_gate = np.zeros((B, NF, LQ), np.float32)
    query_out = np.zeros((B, LQ, D), np.float32)
    for c in range(8):
        b, w = c // 2, c % 2
        r = res[c]
        frag_code[b, w] = np.asarray(r["o_fragcodeT"]).T
        query_code[b, w] = np.asarray(r["o_qcode"])
        frag_self_att[b, w] = np.asarray(r["o_fratt"]).reshape(NFR, LF)
        qf_gate[b, w * NFR:(w + 1) * NFR] = np.asarray(r["o_gate"]).T
        if w == 0:
            query_out[b] = np.asarray(r["o_query"])
    return frag_code, query_code, frag_self_att, qf_gate, query_out
